# revision 84
# baseline (speedup 1.0000x reference)
"""Bidirectional LSTM Trainium2 Bass kernel — gates-transposed layout.

Problem: T=128, B=128, IN=512, H=512, OUT=512 (fp32 reference).
Sharding: data-parallel over batch + direction-parallel:
  cores 0-3: forward LSTM, batch slices 0:32, 32:64, 64:96, 96:128
  cores 4-7: backward LSTM (time-reversed x), same batch slices

Everything lives transposed — gates, c, h are [feature-on-partition,
batch-free] tiles. The recurrent matmul uses W_hh^T blocks as the
STATIONARY operand and h^T (BL=32 columns) as the MOVING operand, so
a step's recurrence costs 32-column matmuls instead of streaming the
512-wide W_hh; phase 1 (xw^T = W_ih^T-blocks @ x^T + bias) accumulates
directly into the same PSUM banks the recurrence continues, and the
cell update produces h^T in place (no transposes at all).

The batch-32 slice is further split into TWO independent 16-column
recurrence chains per core, each with its own PSUM bank per step
([128, 16 gate-tiles, 16] fp32, ring of 2-3) and its own fp16 cell
state, so each chain's serial dependency loop can overlap the other's
engine work. To fit one bank per chain-step, tanh(g) is rewritten as
2*sigmoid(2g)-1 with the 2x folded into the host-side g rows of
W_ih/W_hh/bias — then ONE sigmoid instruction activates all 16 gate
tiles [i f o g']. The per-step chain (the throughput limit) is:
  h(t-1) -> W-MM (fp8e4m3 DoubleRow, K=256/matmul, 0.5 cy/row)
         -> sigmoid(all gates) -> fc, t1=i*g', u=2*t1-i, c=fc+u on
            DVE (fp16, 2x mode) -> tanh(c) -> h-mul -> h(t)
The bf16 h copy for phase 3 runs off-chain on GPSIMD. Phase 1
(xw^T = W_ih^T-blocks @ x^T + a K=16 bias-selection seed) accumulates
one step ahead directly into the recurrence PSUM banks; phase 3
(out^T = W_lin^T-blocks @ h^T bf16) goes per 4-step chunk into its
own PSUM bank, evacuated by DVE and DMA'd per chunk; the last chunk
is split 96+32 columns so only one step's worth of linear remains
after the final cell. Weights DMA per k-tile and the small consts are
packed into one tensor so the first matmuls start ~2us in. Host
combines: out = out_fwd + flip_t(out_bwd) + b_lin.

Numerics: matmuls bf16 except the recurrence (fp8e4m3 both operands),
c in fp16, activations/h in bf16, PSUM accumulation fp32. Measured
rel err 8.6e-3 vs the fp32 reference (tolerance 2e-2).
"""

import sys

sys.path.insert(0, "/opt/trn_rl_repo")

import functools
import os

import ml_dtypes
import numpy as np

import concourse.bass as bass
import concourse.tile as tile
from concourse import bacc, mybir
from concourse.bass_utils import run_bass_kernel_spmd

T, B, IN, H, OUT = 128, 128, 512, 512, 512
NCORES = 8
BL = B // 4  # batch per core (4 cores per direction)
G4 = 4 * H  # 2048 gate rows (transposed: gate-on-partition)
KT = IN // 128  # 4 k-tiles of 128
NGT = G4 // 128  # 16 gate tiles of 128
TCH = T // 4  # 32 column-chunks of 128 (4 steps x 32 batch)
NC_COLS = T * BL  # 4096 (t*32+b) columns

LOOKAHEAD = int(os.environ.get("LSTM_LOOKAHEAD", "1"))
RING = int(os.environ.get("LSTM_RING", "6"))  # psum gates ring (banks)
# half processed FIRST on Act/DVE each step (the other inherits queue lag)
QFIRST = int(os.environ.get("LSTM_QFIRST", "1"))
TC_EARLY = os.environ.get("LSTM_TC_EARLY", "0") == "1"
# merged: one chain per step, gate tiles [i x4, f x4, o x4, g x4], 3 Act insts
MERGED = os.environ.get("LSTM_MERGED", "1") == "1"
FC_POOL = os.environ.get("LSTM_FC_POOL", "0") == "1"
# fp8e4m3 DoubleRow recurrent matmul (W_hh and the recurrence copy of h in
# fp8; phase-3 consumes a separate bf16 h)
FP8WMM = os.environ.get("LSTM_FP8WMM", "1") == "1"

BF16 = mybir.dt.bfloat16
FP16 = mybir.dt.float16
FP32 = mybir.dt.float32
FP8 = mybir.dt.float8e4
AF = mybir.ActivationFunctionType
DROW = mybir.MatmulPerfMode.DoubleRow


def build_nc(reps=1):
    nc = bacc.Bacc(None, target_bir_lowering=False)
    xT = nc.dram_tensor("xT", [128, KT, NC_COLS], BF16, kind="ExternalInput")
    wihT = nc.dram_tensor("wihT", [128, KT, G4], BF16, kind="ExternalInput")
    whhT = nc.dram_tensor("whhT", [128, KT, G4], FP8 if FP8WMM else BF16,
                          kind="ExternalInput")
    wlinT = nc.dram_tensor("wlinT", [128, KT, OUT], BF16, kind="ExternalInput")
    if MERGED:
        # packed small consts, one DMA: [bias16 | sel16c] =
        # [0:128 | 128:384]; sel16c = kron(I16, ones(1,16))
        cpack = nc.dram_tensor("cpack", [16, 384], BF16, kind="ExternalInput")
    else:
        biasm = nc.dram_tensor("biasm", [16, 128], BF16, kind="ExternalInput")
        sel16 = nc.dram_tensor("sel16", [16, NGT * BL], BF16, kind="ExternalInput")
    outp = nc.dram_tensor("outp", [128, 4, NC_COLS], FP32, kind="ExternalOutput")
    debug_t0 = os.environ.get("LSTM_DEBUG_T0") == "1"
    if debug_t0:
        dbg_gates = nc.dram_tensor("dbg_gates", [128, NGT, BL], FP32, kind="ExternalOutput")
        dbg_h = nc.dram_tensor("dbg_h", [128, KT, BL], FP32, kind="ExternalOutput")

    with tile.TileContext(nc) as tc:
        with (
            tc.tile_pool(name="const", bufs=1) as constp,
            tc.tile_pool(name="xring", bufs=4) as xring,
            tc.tile_pool(name="acts", bufs=3) as actsp,
            tc.tile_pool(name="tmps", bufs=2) as tmpsp,
            tc.tile_pool(name="outsb", bufs=3) as outsbp,
            tc.tile_pool(
                name="gates", bufs=(3 if MERGED else RING), space="PSUM"
            ) as gatesp,
            tc.tile_pool(name="ps3", bufs=2, space="PSUM") as ps3,
        ):
            # small consts first (the seed matmuls need them immediately),
            # then weights split per k-tile so phase 1 / the recurrence can
            # start as soon as their first k-slice lands (deps are
            # tile-granular)
            if MERGED:
                cpack_sb = constp.tile([16, 384], BF16)
                nc.sync.dma_start(cpack_sb[:], cpack[:])
                bias16_sb = cpack_sb[:, 0:128]
                sel16c_sb = cpack_sb[:, 128:384]
            else:
                biasm_sb = constp.tile([16, 128], BF16)
                nc.sync.dma_start(biasm_sb[:], biasm[:])
                sel16_sb = constp.tile([16, NGT * BL], BF16)
                nc.sync.dma_start(sel16_sb[:], sel16[:])
            wih_k = [
                constp.tile([128, G4], BF16, name=f"wihk{k}") for k in range(KT)
            ]
            whh_dt = FP8 if FP8WMM else BF16
            whh_j = [
                constp.tile([128, 2, G4], whh_dt, name=f"whhj{j}")
                for j in range(KT // 2)
            ]
            wlin_sb = constp.tile([128, KT, OUT], BF16)
            # h^T history: [128, k-tile, t*32+b]; written per (half, step),
            # read by next step's W-MMs and by phase 3 (subtile deps).
            hT_sb = constp.tile([128, KT, NC_COLS], BF16)
            # fp8 copy of h for the DoubleRow recurrent matmul
            hT_f8 = (
                constp.tile([128, KT, NC_COLS], FP8, name="hT_f8")
                if FP8WMM
                else None
            )
            if MERGED:
                # fp16 cell state per batch-chain: 2-byte dtype enables the
                # DVE 2x_1p mode on the fc/t1/u/c TensorTensor chain
                CB = BL // 2  # 16 batch columns per chain
                c_half = [
                    constp.tile([128, 4, CB], FP16, name=f"c{ch}") for ch in range(2)
                ]
            else:
                c_half = [
                    constp.tile([128, 2, BL], FP32, name=f"c{q}") for q in range(2)
                ]

            for _rep in range(reps):
                for cq in c_half:
                    nc.vector.memset(cq[:], 0.0)
                banks = {}
                xch_tiles = {}

                def ensure_xchunk(ch):
                    if ch not in xch_tiles:
                        xt = xring.tile([128, KT, 128], BF16, tag="xch", name="xch")
                        nc.sync.dma_start(xt[:], xT[:, :, 128 * ch : 128 * ch + 128])
                        xch_tiles[ch] = xt
                    return xch_tiles[ch]

                if _rep == 0:
                    # DMA issue order = arrival order on the serial queue:
                    # first x chunk 0 + wih k0 (unblocks ph1), then the rest
                    # in first-use order; wlin (first used at t=6) last.
                    ensure_xchunk(0)
                    nc.sync.dma_start(wih_k[0][:], wihT[:, 0])
                    for k in range(1, KT):
                        nc.sync.dma_start(wih_k[k][:], wihT[:, k])
                    for j in range(KT // 2):
                        nc.sync.dma_start(whh_j[j][:], whhT[:, 2 * j : 2 * j + 2])
                    nc.sync.dma_start(wlin_sb[:], wlinT[:])

                def emit_ph1(s):
                    ch, ti = s // 4, s % 4
                    xt = ensure_xchunk(ch)
                    # one start=True seed per PSUM bank (start zeroes the
                    # whole bank, so exactly one per bank)
                    if MERGED:
                        bank = []
                        for cn in range(2):
                            bk = gatesp.tile(
                                [128, NGT, CB], FP32, tag=f"bk{cn}", name=f"bk{cn}"
                            )
                            bank.append(bk)
                            nc.tensor.matmul(
                                bk[:],
                                bias16_sb[:],
                                sel16c_sb[:],
                                start=True,
                                stop=False,
                                skip_group_check=True,
                            )
                            c0 = 32 * ti + CB * cn
                            for k in range(KT):
                                for gt in range(NGT):
                                    nc.tensor.matmul(
                                        bk[:, gt, :],
                                        wih_k[k][:, 128 * gt : 128 * gt + 128],
                                        xt[:, k, c0 : c0 + CB],
                                        start=False,
                                        stop=(s == 0 and k == KT - 1),
                                        skip_group_check=True,
                                    )
                        banks[s] = bank
                        return
                    bank = gatesp.tile([128, NGT, BL], FP32, tag="bank", name="bank")
                    nc.tensor.matmul(
                        bank[:],
                        biasm_sb[:],
                        sel16_sb[:],
                        start=True,
                        stop=False,
                        skip_group_check=True,
                    )
                    banks[s] = bank
                    for k in range(KT):
                        for gt in range(NGT):
                            nc.tensor.matmul(
                                bank[:, gt, :],
                                wih_k[k][:, 128 * gt : 128 * gt + 128],
                                xt[:, k, 32 * ti : 32 * ti + 32],
                                start=False,
                                stop=(s == 0 and k == KT - 1),
                                skip_group_check=True,
                            )

                def emit_wmm(t, cn=0):
                    if MERGED:
                        bk = banks[t][cn]
                        c0 = 32 * (t - 1) + CB * cn
                        cols = slice(c0, c0 + CB)

                        # fp8e4m3 DoubleRow: one matmul per (gate-tile,
                        # k-pair) contracts K=256 at 0.5 cycles/row
                        for j in range(KT // 2):
                            for gt in range(NGT):
                                nc.tensor.matmul(
                                    bk[:, gt, :],
                                    whh_j[j][:, :, 128 * gt : 128 * gt + 128],
                                    hT_f8[:, 2 * j : 2 * j + 2, cols],
                                    start=False,
                                    stop=(j == KT // 2 - 1),
                                    perf_mode=DROW,
                                    skip_group_check=True,
                                )
                        return
                    bank = banks[t]
                    cols = slice(32 * (t - 1), 32 * (t - 1) + 32)
                    # k-blocks of the half produced EARLY (QFIRST) run first;
                    # within the late half's k-blocks, the QFIRST half's gate
                    # tiles close first so its activations unblock earliest.
                    kA = (2, 3) if QFIRST == 1 else (0, 1)  # hT of QFIRST
                    kB = (0, 1) if QFIRST == 1 else (2, 3)
                    gF = range(8, NGT) if QFIRST == 1 else range(8)
                    gS = range(8) if QFIRST == 1 else range(8, NGT)
                    korder = [
                        (kA[0], range(NGT)),
                        (kA[1], range(NGT)),
                        (kB[0], gF),
                        (kB[1], gF),
                        (kB[0], gS),
                        (kB[1], gS),
                    ]
                    for k, gts in korder:
                        for gt in gts:
                            nc.tensor.matmul(
                                bank[:, gt, :],
                                whh_j[k // 2][:, k % 2, 128 * gt : 128 * gt + 128],
                                hT_sb[:, k, cols],
                                start=False,
                                stop=(k == KT - 1),
                                skip_group_check=True,
                            )

                def emit_cell(t, cn=0):
                    if MERGED:
                        bk = banks[t][cn]
                        if cn == 1:
                            banks.pop(t)
                        cq = c_half[cn]
                        ahm = actsp.tile(
                            [128, NGT, CB], BF16, tag=f"ahm{cn}", name=f"ahm{cn}"
                        )
                        tcm = actsp.tile(
                            [128, 4, CB], BF16, tag=f"tcm{cn}", name=f"tcm{cn}"
                        )
                        fcm = tmpsp.tile(
                            [128, 4, CB], FP16, tag=f"fcm{cn}", name=f"fcm{cn}"
                        )
                        t1m = tmpsp.tile(
                            [128, 4, CB], FP16, tag=f"t1m{cn}", name=f"t1m{cn}"
                        )
                        um = tmpsp.tile(
                            [128, 4, CB], FP16, tag=f"um{cn}", name=f"um{cn}"
                        )
                        # gate tiles [i f o g'], all sigmoid: tanh(g) was
                        # rewritten as 2*sigmoid(2g)-1 with the 2x folded
                        # into the host-side g rows of W_ih/W_hh/bias
                        nc.scalar.activation(ahm[:], bk[:], AF.Sigmoid)
                        nc.vector.tensor_mul(fcm[:], ahm[:, 4:8, :], cq[:])
                        nc.vector.tensor_mul(t1m[:], ahm[:, 0:4, :], ahm[:, 12:16, :])
                        # u = 2*t1 - sig(i)  ->  i*(2*sig(2g)-1) = i*tanh(g)
                        nc.vector.scalar_tensor_tensor(
                            um[:],
                            t1m[:],
                            2.0,
                            ahm[:, 0:4, :],
                            mybir.AluOpType.mult,
                            mybir.AluOpType.subtract,
                        )
                        nc.vector.tensor_add(cq[:], fcm[:], um[:])
                        nc.scalar.activation(tcm[:], cq[:], AF.Tanh)
                        c0 = 32 * t + CB * cn
                        # chain-critical fp8 h for the recurrence; bf16 h
                        # for phase 3 computed off-chain on GPSIMD
                        nc.vector.tensor_mul(
                            hT_f8[:, :, c0 : c0 + CB], ahm[:, 8:12, :], tcm[:]
                        )
                        nc.gpsimd.tensor_mul(
                            hT_sb[:, :, c0 : c0 + CB], ahm[:, 8:12, :], tcm[:]
                        )
                        return
                    bank = banks.pop(t)
                    if debug_t0 and t == 0:
                        gsb = constp.tile([128, NGT, BL], FP32, name="gsb")
                        nc.vector.tensor_copy(gsb[:], bank[:])
                        nc.sync.dma_start(dbg_gates[:], gsb[:])
                    ah, ag, tct = {}, {}, {}
                    qorder = (QFIRST, 1 - QFIRST)

                    def q_head(q):
                        ah[q] = actsp.tile(
                            [128, 6, BL], BF16, tag=f"ah{q}", name=f"ah{q}"
                        )
                        ag[q] = actsp.tile(
                            [128, 2, BL], BF16, tag=f"ag{q}", name=f"ag{q}"
                        )
                        tct[q] = actsp.tile(
                            [128, 2, BL], BF16, tag=f"tc{q}", name=f"tc{q}"
                        )
                        fc = tmpsp.tile([128, 2, BL], FP32, tag=f"fc{q}", name=f"fc{q}")
                        ig = tmpsp.tile([128, 2, BL], FP32, tag=f"ig{q}", name=f"ig{q}")
                        nc.scalar.activation(
                            ag[q][:], bank[:, 8 * q + 6 : 8 * q + 8, :], AF.Tanh
                        )
                        nc.scalar.activation(
                            ah[q][:], bank[:, 8 * q : 8 * q + 6, :], AF.Sigmoid
                        )
                        # fc on GPSIMD in parallel with ig on DVE
                        if FC_POOL:
                            nc.gpsimd.tensor_mul(fc[:], ah[q][:, 2:4, :], c_half[q][:])
                        else:
                            nc.vector.tensor_mul(fc[:], ah[q][:, 2:4, :], c_half[q][:])
                        nc.vector.tensor_mul(ig[:], ah[q][:, 0:2, :], ag[q][:])
                        nc.vector.tensor_add(c_half[q][:], fc[:], ig[:])

                    def q_tail(q):
                        nc.scalar.activation(tct[q][:], c_half[q][:], AF.Tanh)
                        nc.vector.tensor_mul(
                            hT_sb[:, 2 * q : 2 * q + 2, 32 * t : 32 * t + 32],
                            ah[q][:, 4:6, :],
                            tct[q][:],
                        )

                    if TC_EARLY:
                        q_head(qorder[0])
                        q_tail(qorder[0])
                        q_head(qorder[1])
                        q_tail(qorder[1])
                    else:
                        q_head(qorder[0])
                        q_head(qorder[1])
                        q_tail(qorder[0])
                        q_tail(qorder[1])

                def emit_ph3(ch, c0=0, c1=128):
                    w = c1 - c0
                    po = ps3.tile([128, 4, w], FP32, tag="po", name="po")
                    cols = slice(128 * ch + c0, 128 * ch + c1)
                    for ot in range(4):
                        for k in range(KT):
                            nc.tensor.matmul(
                                po[:, ot, :],
                                wlin_sb[:, k, 128 * ot : 128 * ot + 128],
                                hT_sb[:, k, cols],
                                start=(ot == 0 and k == 0),
                                stop=(k == KT - 1),
                                skip_group_check=True,
                            )
                    ob = outsbp.tile([128, 4, w], FP32, tag="ob", name="ob")
                    nc.vector.tensor_copy(ob[:], po[:])
                    nc.sync.dma_start(outp[:, :, cols], ob[:])

                for s in range(LOOKAHEAD):
                    emit_ph1(s)
                for t in range(T):
                    if debug_t0 and t == 1:
                        hsb = constp.tile([128, KT, BL], FP32, name="hsb")
                        nc.vector.tensor_copy(hsb[:], hT_sb[:, :, 0:BL])
                        nc.sync.dma_start(dbg_h[:], hsb[:])
                    for cn in range(2 if MERGED else 1):
                        if t > 0:
                            emit_wmm(t, cn)
                        emit_cell(t, cn)
                    if t + LOOKAHEAD < T:
                        emit_ph1(t + LOOKAHEAD)
                    if t % 4 == 2 and t >= 4:
                        emit_ph3(t // 4 - 1)
                    if t == T - 1:
                        # first 3 steps of the last chunk: overlaps the
                        # final cell chain
                        emit_ph3(TCH - 1, 0, 96)
                # only the last step's 32 columns remain after h(T-1)
                emit_ph3(TCH - 1, 96, 128)
    nc.compile()
    return nc


@functools.lru_cache(maxsize=1)
def _program():
    return build_nc()


def _gate_perm():
    # PyTorch gate row order: i (0:H), f (H:2H), g (2H:3H), o (3H:4H).
    # Non-merged: per half h tiles [i(2h) i(2h+1) f f o o g g].
    # Merged: tiles [i0 i1 i2 i3 f0..f3 o0..o3 g0..g3].
    off = {"i": 0, "f": H, "g": 2 * H, "o": 3 * H}
    perm = []
    if MERGED:
        for gate in ("i", "f", "o", "g"):
            perm += list(range(off[gate], off[gate] + H))
    else:
        for h in range(2):
            for gate in ("i", "f", "o", "g"):
                for j in (2 * h, 2 * h + 1):
                    perm += list(
                        range(off[gate] + 128 * j, off[gate] + 128 * j + 128)
                    )
    return np.asarray(perm)


def _prep_core(x, W_ih, W_hh, b_ih, b_hh, W_lin, direction, bs):
    perm = _gate_perm()
    bf16 = ml_dtypes.bfloat16
    xs = np.asarray(x)[:, bs : bs + BL, :]
    if direction == 1:
        xs = xs[::-1]
    # xT[p, k, t*32+b] = xs[t, b, 128k+p]
    xTl = np.ascontiguousarray(
        xs.reshape(T, BL, KT, 128).transpose(3, 2, 0, 1).reshape(128, KT, NC_COLS)
    ).astype(bf16)
    Wp_ih = np.asarray(W_ih)[perm].astype(np.float32)  # [G4, IN]
    Wp_hh = np.asarray(W_hh)[perm].astype(np.float32)  # [G4, H]
    bp = (np.asarray(b_ih) + np.asarray(b_hh))[perm].astype(np.float32)
    if MERGED:
        # tanh(g) = 2*sigmoid(2g) - 1: fold the 2x into the g rows
        Wp_ih = Wp_ih.copy()
        Wp_hh = Wp_hh.copy()
        bp = bp.copy()
        Wp_ih[1536:2048] *= 2.0
        Wp_hh[1536:2048] *= 2.0
        bp[1536:2048] *= 2.0
    wihT = np.ascontiguousarray(
        Wp_ih.T.reshape(KT, 128, G4).transpose(1, 0, 2)
    ).astype(bf16)
    whhT = np.ascontiguousarray(
        Wp_hh.T.reshape(KT, 128, G4).transpose(1, 0, 2)
    ).astype(ml_dtypes.float8_e4m3 if FP8WMM else bf16)
    # bias seed matmuls: out[p, gt_local, b] = bias[128*gt + p]
    Wl = np.asarray(W_lin)[:, direction * H : (direction + 1) * H]  # [OUT, H]
    wlinT = np.ascontiguousarray(
        Wl.T.reshape(KT, 128, OUT).transpose(1, 0, 2)
    ).astype(bf16)
    out = {"xT": xTl, "wihT": wihT, "whhT": whhT, "wlinT": wlinT}
    if MERGED:
        cpk = np.zeros((16, 384), np.float32)
        cpk[:, 0:128] = bp.reshape(16, 128)
        cpk[:, 128:384] = np.repeat(np.eye(16, dtype=np.float32), BL // 2, 1)
        out["cpack"] = cpk.astype(bf16)
    else:
        out["biasm"] = np.ascontiguousarray(bp.reshape(16, 128)).astype(bf16)
        out["sel16"] = np.ascontiguousarray(np.repeat(np.eye(16, dtype=bf16), BL, 1))
    return out


def run_cores(inputs, trace=False):
    """Build per-core in_maps, run on 8 cores, return BassKernelResults."""
    in_maps = []
    for core in range(NCORES):
        direction = core // 4
        bs = (core % 4) * BL
        wk = "f" if direction == 0 else "b"
        in_maps.append(
            _prep_core(
                inputs["x"],
                inputs[f"W_ih_{wk}"],
                inputs[f"W_hh_{wk}"],
                inputs[f"b_ih_{wk}"],
                inputs[f"b_hh_{wk}"],
                inputs["W_lin"],
                direction,
                bs,
            )
        )
    nc = _program()
    return run_bass_kernel_spmd(nc, in_maps, list(range(NCORES)), trace=trace)


def _assemble(results, b_lin):
    # per-core outp: [128, 4, T*BL]; part[t, b, 128*ot+p] = outp[p, ot, 32t+b]
    out = np.zeros((T, B, OUT), np.float32)
    for core in range(NCORES):
        direction = core // 4
        bs = (core % 4) * BL
        dev = np.asarray(results[core]["outp"], np.float32)  # [128, 4, 4096]
        part = dev.reshape(128, 4, T, BL).transpose(2, 3, 1, 0).reshape(T, BL, OUT)
        if direction == 1:
            part = part[::-1]
        out[:, bs : bs + BL, :] += part
    out += np.asarray(b_lin, np.float32)[None, None, :]
    return out


def kernel(**inputs):
    res = run_cores(inputs, trace=False)
    return _assemble(res.results, inputs["b_lin"])
